# revision 11
# baseline (speedup 1.0000x reference)
"""Trainium2 Bass kernel for nn_CDALayer (squeeze-excitation-style gated MLP).

Computes: y0 = mean(x, axis=(2,3)); tiny cross-linked MLP -> sigmoid gate;
out = x * gate[:, :, None, None].

Strategy: data-parallel over batch (32 -> 4 per core x 8 cores). The DMA bus
(360 B/ns aggregate in the cost model) is the roofline resource, so x and out
stream through HBM as bf16 — out = x * gate tolerates bf16 rounding (~1% rel
err worst-case vs the 2e-2 gate) and the traffic halves vs f32. The MLP's
linear cross-links are folded host-side:
  y1   = relu(y0 @ (w0_1 + w01).T)
  y2   = relu(y1 @ (w0_2 + w12).T + y0 @ w02.T)
  gate = sigmoid(y2 @ (w0_3 + w23).T + y1 @ w13.T + y0 @ w03.T)
and the 1/(H*W) mean scale is folded into every weight that consumes y0,
so the device only needs row sums of x.

Engine split (from TimelineSim per-op costs): ACT owns the per-chunk row-sum
reduces (Copy + accum_out, 1892ns/chunk) with each image's sigmoids slotted
between the NEXT image's reduces so ACT never stalls on the PE chain; DVE
owns the relus and all gated multiplies (tensor_scalar_mul, 594ns/chunk);
PE does the tiny matmuls; Pool just pulls weights over SWDGE. Loads are
issued batch-major before any store so the single DMA-engine resource never
starves between the load and store streams.
"""

import sys

if "/opt/trn_rl_repo" not in sys.path:
    sys.path.insert(0, "/opt/trn_rl_repo")

import ml_dtypes
import numpy as np

import concourse.bacc as bacc
import concourse.tile as tile
from concourse import mybir
from concourse.bass_utils import run_bass_kernel_spmd

N_CORES = 8
B, C, H, W = 32, 256, 64, 64
BPC = B // N_CORES  # 4 images per core
HW = H * W  # 4096
CR = 16
NH = C // 128  # 2 channel halves (partition tiles)
LOAD_CHUNKS = 2
CHW = HW // LOAD_CHUNKS
F32 = mybir.dt.float32
BF16 = mybir.dt.bfloat16
AF = mybir.ActivationFunctionType

_CACHED = {}


def _build_bass(unroll=1):
    nc = bacc.Bacc("TRN2", target_bir_lowering=False, num_devices=N_CORES)

    x_d = nc.declare_dram_parameter("x", [BPC, NH, 128, HW], BF16, isOutput=False)
    # Weights packed into two tensors so every DMA descriptor row is >=512 B
    # (sub-512B lines pay a 2x wire penalty / SDMA read-modify-write).
    # wbig[:, kh, 0:16]=u1T, [:, kh, 16:32]=u2bT, [:, kh, 32:288]=u3cT.
    # wsm[:, 0:16]=u2aT, [:, 16:272]=u3aT, [:, 272:528]=u3bT.
    wbig_d = nc.declare_dram_parameter("wbig", [128, NH, 2 * CR + C], BF16, isOutput=False)
    wsm_d = nc.declare_dram_parameter("wsm", [CR, CR + 2 * C], BF16, isOutput=False)
    out_d = nc.declare_dram_parameter("out", [BPC, NH, 128, HW], BF16, isOutput=True)

    with tile.TileContext(nc) as tc:
        with (
            tc.tile_pool(name="xpool", bufs=BPC * NH) as xpool,
            tc.tile_pool(name="singles", bufs=1) as singles,
            tc.tile_pool(name="small", bufs=2) as small,
            tc.tile_pool(name="psum", bufs=2, space="PSUM") as psum,
        ):
            # Weight loads ride the gpsimd/SWDGE path so the sync/HWDGE path
            # is free to start streaming x immediately. They are the FIRST
            # Pool-queue ops so wbig's descriptor gen finishes early enough
            # for its transfer to ride the dead window before the first
            # x-load reaches the DMA engines.
            wbig_sb = singles.tile([128, NH, 2 * CR + C], BF16, tag="wbig")
            nc.gpsimd.dma_start(out=wbig_sb, in_=wbig_d[:])
            wsm_sb = singles.tile([CR, CR + 2 * C], BF16, tag="wsm")
            nc.gpsimd.dma_start(out=wsm_sb, in_=wsm_d[:])

            # Warm the ACT function table with the set that covers
            # Copy/Relu/Sigmoid so no mid-pipeline table load happens.
            warm = singles.tile([1, 1], F32, tag="warm")
            nc.gpsimd.memset(warm, 0.0)
            nc.scalar.activation(out=warm, in_=warm, func=AF.Sigmoid)
            w1_sb = wbig_sb[:, :, 0:CR]
            w2b_sb = wbig_sb[:, :, CR : 2 * CR]
            w3c_sb = wbig_sb[:, :, 2 * CR :]
            w2a_sb = wsm_sb[:, 0:CR]
            w3a_sb = wsm_sb[:, CR : CR + C]
            w3b_sb = wsm_sb[:, CR + C :]

            # unroll>1 repeats the whole body (bench-only; same output).
            for _it in range(unroll):
                _body(nc, xpool, small, psum, x_d, out_d,
                      w1_sb, w2a_sb, w2b_sb, w3a_sb, w3b_sb, w3c_sb)

    nc.compile()
    return nc


def _mlp(nc, small, psum, b, y0b, w1_sb, w2a_sb, w2b_sb, w3a_sb, w3b_sb, w3c_sb):
    """PE matmuls + DVE relus for image b; returns (g_sb, sig_ops) where
    sig_ops emits the two ACT sigmoids (deferred so the caller can slot them
    between the next image's ACT reduces)."""
    z1_ps = psum.tile([CR, 1], F32, tag="z1", name=f"z1_{b}")
    nc.tensor.matmul(z1_ps, w1_sb[:, 0, :], y0b[:, 0:1], start=True, stop=False)
    nc.tensor.matmul(z1_ps, w1_sb[:, 1, :], y0b[:, 1:2], start=False, stop=True)
    y1_sb = small.tile([CR, 1], BF16, tag="y1", name=f"y1_{b}")
    nc.vector.tensor_scalar_max(out=y1_sb, in0=z1_ps, scalar1=0.0)

    z2_ps = psum.tile([CR, 1], F32, tag="z2", name=f"z2_{b}")
    nc.tensor.matmul(z2_ps, w2a_sb, y1_sb, start=True, stop=False)
    nc.tensor.matmul(z2_ps, w2b_sb[:, 0, :], y0b[:, 0:1], start=False, stop=False)
    nc.tensor.matmul(z2_ps, w2b_sb[:, 1, :], y0b[:, 1:2], start=False, stop=True)
    y2_sb = small.tile([CR, 1], BF16, tag="y2", name=f"y2_{b}")
    nc.vector.tensor_scalar_max(out=y2_sb, in0=z2_ps, scalar1=0.0)

    g_sb = small.tile([128, NH], F32, tag="g", name=f"g_{b}")
    z3_ps = []
    for mh in range(NH):
        ms = slice(mh * 128, (mh + 1) * 128)
        zp = psum.tile([128, 1], F32, tag=f"z3_{mh}", name=f"z3_{b}_{mh}")
        nc.tensor.matmul(zp, w3a_sb[:, ms], y2_sb, start=True, stop=False)
        nc.tensor.matmul(zp, w3b_sb[:, ms], y1_sb, start=False, stop=False)
        nc.tensor.matmul(zp, w3c_sb[:, 0, ms], y0b[:, 0:1], start=False, stop=False)
        nc.tensor.matmul(zp, w3c_sb[:, 1, ms], y0b[:, 1:2], start=False, stop=True)
        z3_ps.append(zp)

    def sig_ops():
        for mh in range(NH):
            nc.scalar.activation(out=g_sb[:, mh : mh + 1], in_=z3_ps[mh],
                                 func=AF.Sigmoid)

    return g_sb, sig_ops


def _body(nc, xpool, small, psum, x_d, out_d,
          w1_sb, w2a_sb, w2b_sb, w3a_sb, w3b_sb, w3c_sb):
    xt = [[None] * NH for _ in range(BPC)]
    gate = [None] * BPC  # per-image (g_sb, chunks_multiplied) state
    pend_sig = None  # previous image's deferred sigmoid emitter

    def emit_muls(b):
        g_sb, _ = gate[b]
        for h in range(NH):
            for c in range(LOAD_CHUNKS):
                cs = slice(c * CHW, (c + 1) * CHW)
                t = xt[b][h]
                nc.vector.tensor_scalar_mul(
                    out=t[:, cs], in0=t[:, cs], scalar1=g_sb[:, h : h + 1])

    for b in range(BPC):
        pt = small.tile([128, NH, LOAD_CHUNKS], F32, tag="part",
                        name=f"part_{b}")
        # Loads for image b (SP queue, batch-major before any store). Chunk
        # reduces split across engines: h0 on DVE (reduce_sum), h1 on ACT
        # (Copy + accum_out) so neither engine falls behind the 1456ns/chunk
        # DMA arrival rate.
        for h in range(NH):
            t = xpool.tile([128, HW], BF16, tag="xbig", name=f"x_{b}_{h}")
            xt[b][h] = t
            for c in range(LOAD_CHUNKS):
                cs = slice(c * CHW, (c + 1) * CHW)
                nc.sync.dma_start(out=t[:, cs], in_=x_d[b, h, :, cs])
                if h == 0:
                    nc.vector.reduce_sum(
                        out=pt[:, h, c : c + 1], in_=t[:, cs],
                        axis=mybir.AxisListType.X,
                    )
                else:
                    nc.scalar.activation(
                        out=t[:, cs], in_=t[:, cs], func=AF.Copy,
                        accum_out=pt[:, h, c : c + 1],
                    )
        # Previous image's sigmoids run after this image's ACT reduces — by
        # then its PE/DVE chain has long finished, so ACT never stalls.
        if pend_sig is not None:
            pend_sig()
            pend_sig = None
        # Combine chunk partials into per-image channel sums (f32), then
        # cast to bf16 for the PE (bf16 weights halve the weight DMA).
        y0f = small.tile([128, NH], F32, tag="y0f", name=f"y0f_{b}")
        nc.vector.reduce_sum(
            out=y0f[:, 0:1], in_=pt[:, 0, :], axis=mybir.AxisListType.X)
        nc.scalar.activation(
            out=pt[:, 1, :], in_=pt[:, 1, :], func=AF.Copy,
            accum_out=y0f[:, 1:2],
        )
        y0b = small.tile([128, NH], BF16, tag="y0b", name=f"y0b_{b}")
        nc.scalar.activation(out=y0b, in_=y0f, func=AF.Copy)

        # Previous image's gated multiplies go ahead of this image's relus in
        # the DVE queue (gate b-1 is ready long before relu b's inputs).
        if b > 0:
            emit_muls(b - 1)

        g_sb, sig = _mlp(nc, small, psum, b, y0b,
                         w1_sb, w2a_sb, w2b_sb, w3a_sb, w3b_sb, w3c_sb)
        gate[b] = (g_sb, False)
        pend_sig = sig

    pend_sig()  # last image's sigmoids (ACT is idle by now)
    emit_muls(BPC - 1)

    # Stores issue strictly after every load on the SP queue; each waits only
    # on its chunk's multiply.
    for b in range(BPC):
        for h in range(NH):
            t = xt[b][h]
            for c in range(LOAD_CHUNKS):
                cs = slice(c * CHW, (c + 1) * CHW)
                nc.sync.dma_start(out=out_d[b, h, :, cs], in_=t[:, cs])


def _prep_weights(w0_1, w0_2, w0_3, w01, w02, w03, w12, w13, w23):
    inv = np.float32(1.0 / HW)
    u1 = (w0_1 + w01) * inv  # [CR, C], consumes y0 sums
    u2a = w0_2 + w12  # [CR, CR]
    u2b = w02 * inv  # [CR, C]
    u3a = w0_3 + w23  # [C, CR]
    u3b = w13  # [C, CR]
    u3c = w03 * inv  # [C, C]

    def t_khalf(u):  # [out, C] -> lhsT layout [128, NH, out]
        return np.ascontiguousarray(
            u.T.reshape(NH, 128, u.shape[0]).transpose(1, 0, 2)
        ).astype(np.float32)

    wbig = np.concatenate([t_khalf(u1), t_khalf(u2b), t_khalf(u3c)], axis=2)
    wsm = np.concatenate(
        [np.ascontiguousarray(u2a.T), np.ascontiguousarray(u3a.T),
         np.ascontiguousarray(u3b.T)], axis=1)
    return {
        "wbig": np.ascontiguousarray(wbig).astype(ml_dtypes.bfloat16),
        "wsm": np.ascontiguousarray(wsm).astype(ml_dtypes.bfloat16),
    }


def kernel(run_opts=None, **inputs):
    x = np.asarray(inputs["x"], dtype=np.float32)
    assert x.shape == (B, C, H, W), x.shape

    weights = _prep_weights(
        *(np.asarray(inputs[k], dtype=np.float32)
          for k in ("w0_1", "w0_2", "w0_3", "w01", "w02", "w03", "w12", "w13", "w23"))
    )

    if "nc" not in _CACHED:
        _CACHED["nc"] = _build_bass()
    nc = _CACHED["nc"]

    # x and out stream through HBM as bf16 (see module docstring).
    xv = x.reshape(B, NH, 128, HW).astype(ml_dtypes.bfloat16)
    in_maps = [
        {"x": xv[c * BPC : (c + 1) * BPC], **weights} for c in range(N_CORES)
    ]
    # The axon terminal occasionally throws a transient device error
    # (e.g. NRT_EXEC_UNIT_UNRECOVERABLE); each run is independent with
    # fresh output buffers, so a short retry is safe.
    last_exc = None
    for attempt in range(3):
        try:
            res = run_bass_kernel_spmd(nc, in_maps, core_ids=list(range(N_CORES)),
                                       **(run_opts or {}))
            break
        except Exception as e:
            last_exc = e
            import time
            time.sleep(5 * (attempt + 1))
    else:
        raise last_exc
    out = np.concatenate(
        [r["out"].reshape(BPC, C, H, W).astype(np.float32) for r in res.results],
        axis=0,
    )
    if run_opts:
        _CACHED["last_result"] = res
    return out


# revision 14
# speedup vs baseline: 1.0046x; 1.0046x over previous
"""Trainium2 Bass kernel for nn_CDALayer (squeeze-excitation-style gated MLP).

Computes: y0 = mean(x, axis=(2,3)); tiny cross-linked MLP -> sigmoid gate;
out = x * gate[:, :, None, None].

Strategy: data-parallel over batch (32 -> 4 per core x 8 cores). The DMA bus
(360 B/ns aggregate in the cost model) is the roofline resource, so x and out
stream through HBM as bf16 — out = x * gate tolerates bf16 rounding (~1% rel
err worst-case vs the 2e-2 gate) and the traffic halves vs f32. The MLP's
linear cross-links are folded host-side:
  y1   = relu(y0 @ (w0_1 + w01).T)
  y2   = relu(y1 @ (w0_2 + w12).T + y0 @ w02.T)
  gate = sigmoid(y2 @ (w0_3 + w23).T + y1 @ w13.T + y0 @ w03.T)
and the 1/(H*W) mean scale is folded into every weight that consumes y0,
so the device only needs row sums of x.

Engine split (from TimelineSim per-op costs): ACT owns the per-chunk row-sum
reduces (Copy + accum_out, 1892ns/chunk) with each image's sigmoids slotted
between the NEXT image's reduces so ACT never stalls on the PE chain; DVE
owns the relus and all gated multiplies (tensor_scalar_mul, 594ns/chunk);
PE does the tiny matmuls; Pool just pulls weights over SWDGE. Loads are
issued batch-major before any store so the single DMA-engine resource never
starves between the load and store streams.
"""

import sys

if "/opt/trn_rl_repo" not in sys.path:
    sys.path.insert(0, "/opt/trn_rl_repo")

import ml_dtypes
import numpy as np

import concourse.bacc as bacc
import concourse.tile as tile
from concourse import mybir
from concourse.bass_utils import run_bass_kernel_spmd

N_CORES = 8
B, C, H, W = 32, 256, 64, 64
BPC = B // N_CORES  # 4 images per core
HW = H * W  # 4096
CR = 16
NH = C // 128  # 2 channel halves (partition tiles)
LOAD_CHUNKS = 2
CHW = HW // LOAD_CHUNKS
F32 = mybir.dt.float32
BF16 = mybir.dt.bfloat16
FP8 = mybir.dt.float8e4
AF = mybir.ActivationFunctionType

_CACHED = {}


def _build_bass(unroll=1):
    nc = bacc.Bacc("TRN2", target_bir_lowering=False, num_devices=N_CORES)

    x_d = nc.declare_dram_parameter("x", [BPC, NH, 128, HW], BF16, isOutput=False)
    # Weights packed into two fp8 tensors so every DMA descriptor row is
    # >=512 B (sub-512B lines pay a 2x wire penalty): wbig rows are
    # NH*288 = 576 B, wsm rows 528 B. fp8 requires the 1/(H*W) scale NOT be
    # folded into the weights (folded values ~1e-5 would flush to zero);
    # instead y0 is scaled during its fp8 cast.
    # wbig[:, kh, 0:16]=u1T, [:, kh, 16:32]=u2bT, [:, kh, 32:288]=u3cT.
    # wsm[:, 0:16]=u2aT, [:, 16:272]=u3aT, [:, 272:528]=u3bT.
    wbig_d = nc.declare_dram_parameter("wbig", [128, NH, 2 * CR + C], FP8, isOutput=False)
    wsm_d = nc.declare_dram_parameter("wsm", [CR, CR + 2 * C], FP8, isOutput=False)
    out_d = nc.declare_dram_parameter("out", [BPC, NH, 128, HW], BF16, isOutput=True)

    with tile.TileContext(nc) as tc:
        with (
            tc.tile_pool(name="xpool", bufs=BPC * NH) as xpool,
            tc.tile_pool(name="small", bufs=2) as small,
            tc.tile_pool(name="psum", bufs=2, space="PSUM") as psum,
        ):
            # Weight loads ride the gpsimd/SWDGE path so the sync/HWDGE path
            # is free to start streaming x immediately.
            wbig_sb = small.tile([128, NH, 2 * CR + C], FP8, tag="wbig")
            nc.gpsimd.dma_start(out=wbig_sb, in_=wbig_d[:])
            wsm_sb = small.tile([CR, CR + 2 * C], FP8, tag="wsm")
            nc.gpsimd.dma_start(out=wsm_sb, in_=wsm_d[:])

            # Warm the ACT function table with the set that covers
            # Copy/Relu/Sigmoid so no mid-pipeline table load happens.
            warm = small.tile([1, 1], F32, tag="warm")
            nc.gpsimd.memset(warm, 0.0)
            nc.scalar.activation(out=warm, in_=warm, func=AF.Sigmoid)
            w1_sb = wbig_sb[:, :, 0:CR]
            w2b_sb = wbig_sb[:, :, CR : 2 * CR]
            w3c_sb = wbig_sb[:, :, 2 * CR :]
            w2a_sb = wsm_sb[:, 0:CR]
            w3a_sb = wsm_sb[:, CR : CR + C]
            w3b_sb = wsm_sb[:, CR + C :]

            # unroll>1 repeats the whole body (bench-only; same output).
            for _it in range(unroll):
                _body(nc, xpool, small, psum, x_d, out_d,
                      w1_sb, w2a_sb, w2b_sb, w3a_sb, w3b_sb, w3c_sb)

    nc.compile()
    return nc


def _mlp(nc, small, psum, b, y0b, w1_sb, w2a_sb, w2b_sb, w3a_sb, w3b_sb, w3c_sb):
    """PE matmuls + DVE relus for image b; returns (g_sb, sig_ops) where
    sig_ops emits the two ACT sigmoids (deferred so the caller can slot them
    between the next image's ACT reduces)."""
    z1_ps = psum.tile([CR, 1], F32, tag="z1", name=f"z1_{b}")
    nc.tensor.matmul(z1_ps, w1_sb[:, 0, :], y0b[:, 0:1], start=True, stop=False)
    nc.tensor.matmul(z1_ps, w1_sb[:, 1, :], y0b[:, 1:2], start=False, stop=True)
    y1_sb = small.tile([CR, 1], FP8, tag="y1", name=f"y1_{b}")
    nc.vector.tensor_scalar_max(out=y1_sb, in0=z1_ps, scalar1=0.0)

    z2_ps = psum.tile([CR, 1], F32, tag="z2", name=f"z2_{b}")
    nc.tensor.matmul(z2_ps, w2a_sb, y1_sb, start=True, stop=False)
    nc.tensor.matmul(z2_ps, w2b_sb[:, 0, :], y0b[:, 0:1], start=False, stop=False)
    nc.tensor.matmul(z2_ps, w2b_sb[:, 1, :], y0b[:, 1:2], start=False, stop=True)
    y2_sb = small.tile([CR, 1], FP8, tag="y2", name=f"y2_{b}")
    nc.vector.tensor_scalar_max(out=y2_sb, in0=z2_ps, scalar1=0.0)

    g_sb = small.tile([128, NH], F32, tag="g", name=f"g_{b}")
    z3_ps = []
    for mh in range(NH):
        ms = slice(mh * 128, (mh + 1) * 128)
        zp = psum.tile([128, 1], F32, tag=f"z3_{mh}", name=f"z3_{b}_{mh}")
        nc.tensor.matmul(zp, w3a_sb[:, ms], y2_sb, start=True, stop=False)
        nc.tensor.matmul(zp, w3b_sb[:, ms], y1_sb, start=False, stop=False)
        nc.tensor.matmul(zp, w3c_sb[:, 0, ms], y0b[:, 0:1], start=False, stop=False)
        nc.tensor.matmul(zp, w3c_sb[:, 1, ms], y0b[:, 1:2], start=False, stop=True)
        z3_ps.append(zp)

    def sig_ops():
        for mh in range(NH):
            nc.scalar.activation(out=g_sb[:, mh : mh + 1], in_=z3_ps[mh],
                                 func=AF.Sigmoid)

    return g_sb, sig_ops


def _body(nc, xpool, small, psum, x_d, out_d,
          w1_sb, w2a_sb, w2b_sb, w3a_sb, w3b_sb, w3c_sb):
    xt = [[None] * NH for _ in range(BPC)]
    gate = [None] * BPC  # per-image (g_sb, chunks_multiplied) state
    pend_sig = None  # previous image's deferred sigmoid emitter

    def emit_muls(b):
        g_sb, _ = gate[b]
        for h in range(NH):
            for c in range(LOAD_CHUNKS):
                cs = slice(c * CHW, (c + 1) * CHW)
                t = xt[b][h]
                nc.vector.tensor_scalar_mul(
                    out=t[:, cs], in0=t[:, cs], scalar1=g_sb[:, h : h + 1])

    for b in range(BPC):
        pt = small.tile([128, NH, LOAD_CHUNKS], F32, tag="part",
                        name=f"part_{b}")
        # Loads for image b (SP queue, batch-major before any store). Chunk
        # reduces split across engines: h0 on DVE (reduce_sum), h1 on ACT
        # (Copy + accum_out) so neither engine falls behind the 1456ns/chunk
        # DMA arrival rate.
        for h in range(NH):
            t = xpool.tile([128, HW], BF16, tag="xbig", name=f"x_{b}_{h}")
            xt[b][h] = t
            for c in range(LOAD_CHUNKS):
                cs = slice(c * CHW, (c + 1) * CHW)
                nc.sync.dma_start(out=t[:, cs], in_=x_d[b, h, :, cs])
                if h == 0:
                    nc.vector.reduce_sum(
                        out=pt[:, h, c : c + 1], in_=t[:, cs],
                        axis=mybir.AxisListType.X,
                    )
                else:
                    nc.scalar.activation(
                        out=t[:, cs], in_=t[:, cs], func=AF.Copy,
                        accum_out=pt[:, h, c : c + 1],
                    )
        # Previous image's sigmoids run after this image's ACT reduces — by
        # then its PE/DVE chain has long finished, so ACT never stalls.
        if pend_sig is not None:
            pend_sig()
            pend_sig = None
        # Combine chunk partials into per-image channel sums (f32), then
        # cast to bf16 for the PE (bf16 weights halve the weight DMA).
        y0f = small.tile([128, NH], F32, tag="y0f", name=f"y0f_{b}")
        nc.vector.reduce_sum(
            out=y0f[:, 0:1], in_=pt[:, 0, :], axis=mybir.AxisListType.X)
        nc.scalar.activation(
            out=pt[:, 1, :], in_=pt[:, 1, :], func=AF.Copy,
            accum_out=y0f[:, 1:2],
        )
        y0b = small.tile([128, NH], FP8, tag="y0b", name=f"y0b_{b}")
        nc.scalar.activation(out=y0b, in_=y0f, func=AF.Copy, scale=1.0 / HW)

        # Previous image's gated multiplies go ahead of this image's relus in
        # the DVE queue (gate b-1 is ready long before relu b's inputs).
        if b > 0:
            emit_muls(b - 1)

        g_sb, sig = _mlp(nc, small, psum, b, y0b,
                         w1_sb, w2a_sb, w2b_sb, w3a_sb, w3b_sb, w3c_sb)
        gate[b] = (g_sb, False)
        pend_sig = sig

    pend_sig()  # last image's sigmoids (ACT is idle by now)
    emit_muls(BPC - 1)

    # Stores issue strictly after every load on the SP queue; each waits only
    # on its chunk's multiply.
    for b in range(BPC):
        for h in range(NH):
            t = xt[b][h]
            for c in range(LOAD_CHUNKS):
                cs = slice(c * CHW, (c + 1) * CHW)
                nc.sync.dma_start(out=out_d[b, h, :, cs], in_=t[:, cs])


def _prep_weights(w0_1, w0_2, w0_3, w01, w02, w03, w12, w13, w23):
    # No 1/(H*W) folding: fp8 can't represent ~1e-5 weights; the scale is
    # applied on-device when y0 is cast to fp8.
    u1 = w0_1 + w01  # [CR, C], consumes y0 means
    u2a = w0_2 + w12  # [CR, CR]
    u2b = w02  # [CR, C]
    u3a = w0_3 + w23  # [C, CR]
    u3b = w13  # [C, CR]
    u3c = w03  # [C, C]

    def t_khalf(u):  # [out, C] -> lhsT layout [128, NH, out]
        return np.ascontiguousarray(
            u.T.reshape(NH, 128, u.shape[0]).transpose(1, 0, 2)
        ).astype(np.float32)

    wbig = np.concatenate([t_khalf(u1), t_khalf(u2b), t_khalf(u3c)], axis=2)
    wsm = np.concatenate(
        [np.ascontiguousarray(u2a.T), np.ascontiguousarray(u3a.T),
         np.ascontiguousarray(u3b.T)], axis=1)
    return {
        "wbig": np.ascontiguousarray(wbig).astype(ml_dtypes.float8_e4m3),
        "wsm": np.ascontiguousarray(wsm).astype(ml_dtypes.float8_e4m3),
    }


def kernel(run_opts=None, **inputs):
    x = np.asarray(inputs["x"], dtype=np.float32)
    assert x.shape == (B, C, H, W), x.shape

    weights = _prep_weights(
        *(np.asarray(inputs[k], dtype=np.float32)
          for k in ("w0_1", "w0_2", "w0_3", "w01", "w02", "w03", "w12", "w13", "w23"))
    )

    if "nc" not in _CACHED:
        _CACHED["nc"] = _build_bass()
    nc = _CACHED["nc"]

    # x and out stream through HBM as bf16 (see module docstring).
    xv = x.reshape(B, NH, 128, HW).astype(ml_dtypes.bfloat16)
    in_maps = [
        {"x": xv[c * BPC : (c + 1) * BPC], **weights} for c in range(N_CORES)
    ]
    # The axon terminal occasionally throws a transient device error
    # (e.g. NRT_EXEC_UNIT_UNRECOVERABLE); each run is independent with
    # fresh output buffers, so a short retry is safe.
    last_exc = None
    for attempt in range(3):
        try:
            res = run_bass_kernel_spmd(nc, in_maps, core_ids=list(range(N_CORES)),
                                       **(run_opts or {}))
            break
        except Exception as e:
            last_exc = e
            import time
            time.sleep(5 * (attempt + 1))
    else:
        raise last_exc
    out = np.concatenate(
        [r["out"].reshape(BPC, C, H, W).astype(np.float32) for r in res.results],
        axis=0,
    )
    if run_opts:
        _CACHED["last_result"] = res
    return out
